# revision 7
# baseline (speedup 1.0000x reference)
"""LocallyConnected1D (B=8, L=4096, C=64, K=3, F=64) on 8 TRN2 NeuronCores.

out[b, l, f] = sum_{k,c} x[b, l+k, c] * kernel[l, k, c, f] + bias[l, f]

Strategy (spatial sharding, 512 output positions per core):
  - For each pair of adjacent output positions (l0+2i, l0+2i+1) build a
    block-diagonal stationary tile lhsT (128 x 16): partitions = 2 phases x 64
    channels, columns = 2 phases x 8 batch.  Streaming operand = the pair's
    per-position weights (128 x 64).  Three PSUM-accumulated matmuls per pair
    (one per tap k, using x-pair tiles shifted by k) produce out (16, 64).
  - Weights are rearranged on host to (128, 256, 3, 64) per core so the 25 MB
    weight stream is one contiguous DMA per block -> full HBM bandwidth.
  - Bias is added on host (it is tiny); dtype is preserved (f32 in/out).
"""

import numpy as np

import concourse.bass as bass
import concourse.mybir as mybir
import concourse.tile as tile
from concourse import bacc
from concourse.bass import ds, ts
from concourse.bass_utils import run_bass_kernel_spmd

B, L, C, K, F = 8, 4096, 64, 3, 64
L_OUT = (L - K) + 1  # 4094
N_CORES = 8
P_CORE = 512          # output positions per core (last core: 510 real + 2 pad)
PAIRS = P_CORE // 2   # 256
W_BLK = 32            # pairs per weight DMA block
DT = mybir.dt.float32

_CACHE = {}


def _build_nc():
    nc = bacc.Bacc("TRN2", target_bir_lowering=False, debug=False)

    w_d = nc.declare_dram_parameter("w", [128, PAIRS, K, F], DT, isOutput=False)
    te_d = nc.declare_dram_parameter("te", [128, PAIRS + 1, 16], DT, isOutput=False)
    to_d = nc.declare_dram_parameter("to", [128, PAIRS, 16], DT, isOutput=False)
    # out[g, m, j*64+f]: g = group of 8 pairs (32 total), m = phase*8 + b.
    out_d = nc.declare_dram_parameter("out", [PAIRS // 8, 16, 512], DT, isOutput=True)

    with tile.TileContext(nc) as tc:
        with (
            tc.tile_pool(name="tpool", bufs=1) as tpool,
            tc.tile_pool(name="wpool", bufs=3) as wpool,
            tc.tile_pool(name="opool", bufs=4) as opool,
            tc.tile_pool(name="pspool", bufs=8, space=bass.MemorySpace.PSUM) as pspool,
        ):
            te = tpool.tile([128, PAIRS + 1, 16], DT)
            to = tpool.tile([128, PAIRS, 16], DT)
            nc.sync.dma_start(te[:], te_d[:])
            nc.sync.dma_start(to[:], to_d[:])

            for h in range(PAIRS // W_BLK):  # 8 weight blocks of 32 pairs
                wb = wpool.tile([128, W_BLK, K, F], DT)
                nc.sync.dma_start(wb[:], w_d[:, ds(h * W_BLK, W_BLK)])
                for g2 in range(4):  # groups of 8 pairs -> one PSUM bank
                    g = h * 4 + g2   # global group (0..31)
                    acc = pspool.tile([128, 512], DT)  # one full bank
                    for j in range(8):
                        i = g * 8 + j    # global pair
                        jj = g2 * 8 + j  # pair in W block
                        nc.tensor.matmul(
                            acc[0:16, ts(j, 64)], te[:, i, :], wb[:, jj, 0, :],
                            start=True, stop=False)
                        nc.tensor.matmul(
                            acc[0:16, ts(j, 64)], to[:, i, :], wb[:, jj, 1, :],
                            start=False, stop=False)
                        nc.tensor.matmul(
                            acc[0:16, ts(j, 64)], te[:, i + 1, :], wb[:, jj, 2, :],
                            start=False, stop=True)
                    ob = opool.tile([16, 512], DT)
                    nc.vector.tensor_copy(ob[:], acc[0:16, :])
                    nc.sync.dma_start(out_d[g], ob[:])

    nc.compile()
    return nc


def _prep_inputs(x, kernel):
    """Host-side rearrangement into per-core DMA-friendly layouts."""
    xp = np.zeros((B, L + 4, C), np.float32)
    xp[:, :L] = x
    kp = np.zeros((N_CORES * P_CORE, K, C, F), np.float32)
    kp[:L_OUT] = kernel
    in_maps = []
    for m in range(N_CORES):
        l0 = P_CORE * m
        xs = xp[:, l0:l0 + 2 * PAIRS + 2, :]
        ev = np.ascontiguousarray(xs[:, 0::2].transpose(2, 1, 0))  # (64, 257, 8) j=2i
        od = np.ascontiguousarray(xs[:, 1::2].transpose(2, 1, 0))  # (64, 257, 8) j=2i+1
        TE = np.zeros((128, PAIRS + 1, 16), np.float32)
        TE[:64, :, 0:8] = ev
        TE[64:, :, 8:16] = od
        TO = np.zeros((128, PAIRS, 16), np.float32)
        TO[:64, :, 0:8] = od[:, :PAIRS]
        TO[64:, :, 8:16] = ev[:, 1:PAIRS + 1]
        W = np.ascontiguousarray(
            kp[l0:l0 + P_CORE]
            .reshape(PAIRS, 2, K, C, F)
            .transpose(1, 3, 0, 2, 4)
            .reshape(128, PAIRS, K, F))
        in_maps.append({"w": W, "te": TE, "to": TO})
    return in_maps


def _unpack_out(res):
    """(32,16,512) per core -> (B, P_CORE, F).  l_local = 16g + 2j + phase."""
    return (res.reshape(32, 2, 8, 8, 64)          # [g, phase, b, j, f]
            .transpose(2, 0, 3, 1, 4)              # [b, g, j, phase, f]
            .reshape(B, P_CORE, F))


def kernel(x, kernel, bias):
    x = np.asarray(x, dtype=np.float32)
    kern = np.asarray(kernel, dtype=np.float32)
    bias = np.asarray(bias, dtype=np.float32)

    if "nc" not in _CACHE:
        _CACHE["nc"] = _build_nc()
    nc = _CACHE["nc"]

    in_maps = _prep_inputs(x, kern)
    results = run_bass_kernel_spmd(nc, in_maps, list(range(N_CORES))).results

    parts = [_unpack_out(results[m]["out"]) for m in range(N_CORES)]
    out = np.concatenate(parts, axis=1)[:, :L_OUT]
    return (out + bias[None]).astype(np.float32)


# revision 8
# speedup vs baseline: 26394.0603x; 26394.0603x over previous
"""LocallyConnected1D (B=8, L=4096, C=64, K=3, F=64) on 8 TRN2 NeuronCores.

out[b, l, f] = sum_{k,c} x[b, l+k, c] * kernel[l, k, c, f] + bias[l, f]

Strategy (spatial sharding, 512 output positions per core):
  - For each pair of adjacent output positions (l0+2i, l0+2i+1) build a
    block-diagonal stationary tile lhsT (128 x 16): partitions = 2 phases x 64
    channels, columns = 2 phases x 8 batch.  Streaming operand = the pair's
    per-position weights (128 x 64).  Three PSUM-accumulated matmuls per pair
    (one per tap k, using x-pair tiles shifted by k) produce out (16, 64).
  - Weights are rearranged on host to (128, 256, 3, 64) per core so the 25 MB
    weight stream is one contiguous DMA per block -> full HBM bandwidth.
  - Bias is added on host (it is tiny); dtype is preserved (f32 in/out).
"""

import numpy as np

import concourse.bass as bass
import concourse.mybir as mybir
import concourse.tile as tile
from concourse import bacc
from concourse.bass import ds, ts
from concourse.bass_utils import run_bass_kernel_spmd

B, L, C, K, F = 8, 4096, 64, 3, 64
L_OUT = (L - K) + 1  # 4094
N_CORES = 8
P_CORE = 512          # output positions per core (last core: 510 real + 2 pad)
PAIRS = P_CORE // 2   # 256
W_BLK = 32            # pairs per weight DMA block
DT = mybir.dt.float32

_CACHE = {}


def _build_body(nc, tpool, wpool, opool, pspool, w_d, te_d, to_d, out_d):
    te = tpool.tile([128, PAIRS + 1, 16], DT)
    to = tpool.tile([128, PAIRS, 16], DT)
    nc.sync.dma_start(te[:], te_d[:])
    nc.sync.dma_start(to[:], to_d[:])

    for h in range(PAIRS // W_BLK):  # weight blocks of W_BLK pairs
        wb = wpool.tile([128, W_BLK, K, F], DT)
        nc.sync.dma_start(wb[:], w_d[:, ds(h * W_BLK, W_BLK)])
        for g2 in range(W_BLK // 8):  # groups of 8 pairs -> one PSUM bank
            g = h * (W_BLK // 8) + g2   # global group (0..31)
            acc = pspool.tile([128, 512], DT)  # one full bank
            for j in range(8):
                i = g * 8 + j    # global pair
                jj = g2 * 8 + j  # pair in W block
                nc.tensor.matmul(
                    acc[0:16, ts(j, 64)], te[:, i, :], wb[:, jj, 0, :],
                    start=True, stop=False)
                nc.tensor.matmul(
                    acc[0:16, ts(j, 64)], to[:, i, :], wb[:, jj, 1, :],
                    start=False, stop=False)
                nc.tensor.matmul(
                    acc[0:16, ts(j, 64)], te[:, i + 1, :], wb[:, jj, 2, :],
                    start=False, stop=True)
            ob = opool.tile([16, 512], DT)
            nc.vector.tensor_copy(ob[:], acc[0:16, :])
            nc.sync.dma_start(out_d[g], ob[:])


def _build_nc(n_iters=None):
    """n_iters=None: straight-line kernel (graded path).
    n_iters=N: body wrapped in a HW For_i loop, for timing-slope runs."""
    nc = bacc.Bacc("TRN2", target_bir_lowering=False, debug=False)

    w_d = nc.declare_dram_parameter("w", [128, PAIRS, K, F], DT, isOutput=False)
    te_d = nc.declare_dram_parameter("te", [128, PAIRS + 1, 16], DT, isOutput=False)
    to_d = nc.declare_dram_parameter("to", [128, PAIRS, 16], DT, isOutput=False)
    # out[g, m, j*64+f]: g = group of 8 pairs (32 total), m = phase*8 + b.
    out_d = nc.declare_dram_parameter("out", [PAIRS // 8, 16, 512], DT, isOutput=True)

    with tile.TileContext(nc) as tc:
        with (
            tc.tile_pool(name="tpool", bufs=1) as tpool,
            tc.tile_pool(name="wpool", bufs=3) as wpool,
            tc.tile_pool(name="opool", bufs=4) as opool,
            tc.tile_pool(name="pspool", bufs=8, space=bass.MemorySpace.PSUM) as pspool,
        ):
            if n_iters is None:
                _build_body(nc, tpool, wpool, opool, pspool, w_d, te_d, to_d, out_d)
            else:
                with tc.For_i(0, n_iters, 1):
                    _build_body(nc, tpool, wpool, opool, pspool,
                                w_d, te_d, to_d, out_d)

    nc.compile()
    return nc


def _prep_inputs(x, kernel):
    """Host-side rearrangement into per-core DMA-friendly layouts."""
    xp = np.zeros((B, L + 4, C), np.float32)
    xp[:, :L] = x
    kp = np.zeros((N_CORES * P_CORE, K, C, F), np.float32)
    kp[:L_OUT] = kernel
    in_maps = []
    for m in range(N_CORES):
        l0 = P_CORE * m
        xs = xp[:, l0:l0 + 2 * PAIRS + 2, :]
        ev = np.ascontiguousarray(xs[:, 0::2].transpose(2, 1, 0))  # (64, 257, 8) j=2i
        od = np.ascontiguousarray(xs[:, 1::2].transpose(2, 1, 0))  # (64, 257, 8) j=2i+1
        TE = np.zeros((128, PAIRS + 1, 16), np.float32)
        TE[:64, :, 0:8] = ev
        TE[64:, :, 8:16] = od
        TO = np.zeros((128, PAIRS, 16), np.float32)
        TO[:64, :, 0:8] = od[:, :PAIRS]
        TO[64:, :, 8:16] = ev[:, 1:PAIRS + 1]
        W = np.ascontiguousarray(
            kp[l0:l0 + P_CORE]
            .reshape(PAIRS, 2, K, C, F)
            .transpose(1, 3, 0, 2, 4)
            .reshape(128, PAIRS, K, F))
        in_maps.append({"w": W, "te": TE, "to": TO})
    return in_maps


def _unpack_out(res):
    """(32,16,512) per core -> (B, P_CORE, F).  l_local = 16g + 2j + phase."""
    return (res.reshape(32, 2, 8, 8, 64)          # [g, phase, b, j, f]
            .transpose(2, 0, 3, 1, 4)              # [b, g, j, phase, f]
            .reshape(B, P_CORE, F))


def kernel(x, kernel, bias):
    x = np.asarray(x, dtype=np.float32)
    kern = np.asarray(kernel, dtype=np.float32)
    bias = np.asarray(bias, dtype=np.float32)

    if "nc" not in _CACHE:
        _CACHE["nc"] = _build_nc()
    nc = _CACHE["nc"]

    in_maps = _prep_inputs(x, kern)
    results = run_bass_kernel_spmd(nc, in_maps, list(range(N_CORES))).results

    parts = [_unpack_out(results[m]["out"]) for m in range(N_CORES)]
    out = np.concatenate(parts, axis=1)[:, :L_OUT]
    return (out + bias[None]).astype(np.float32)


# revision 15
# speedup vs baseline: 66168.1275x; 2.5069x over previous
"""LocallyConnected1D (B=8, L=4096, C=64, K=3, F=64) on 8 TRN2 NeuronCores.

out[b, l, f] = sum_{k,c} x[b, l+k, c] * kernel[l, k, c, f] + bias[l, f]

Strategy (spatial sharding, 512 output positions per core):
  - For each pair of adjacent output positions (l0+2i, l0+2i+1) build a
    block-diagonal stationary tile lhsT (128 x 16): partitions = 2 phases x 64
    channels, columns = 2 phases x 8 batch.  Streaming operand = the pair's
    per-position weights (128 x 64).  Three PSUM-accumulated matmuls per pair
    (one per tap k, using x-pair tiles shifted by k) produce out (16, 64).
  - Groups of 8 pairs are dispatched to 4 independent 32-column strips of the
    PE array (tile_position), each strip accumulating into its own PSUM bank,
    so up to 4 matmuls run concurrently in the array.
  - Weights are rearranged on host to (128, 256, 3, 64) per core so the
    weight stream is one contiguous DMA per block -> full HBM bandwidth.
  - Bias is added on host (it is tiny); output dtype f32.
"""

import numpy as np
import ml_dtypes

import concourse.bass as bass
import concourse.mybir as mybir
import concourse.tile as tile
from concourse import bacc
from concourse.bass import ds, ts
from concourse.bass_utils import run_bass_kernel_spmd

B, L, C, K, F = 8, 4096, 64, 3, 64
L_OUT = (L - K) + 1  # 4094
N_CORES = 8
P_CORE = 512          # output positions per core (last core: 510 real + 2 pad)
PAIRS = P_CORE // 2   # 256
W_BLK = 32            # pairs per weight DMA block

USE_BF16 = True
DT = mybir.dt.bfloat16 if USE_BF16 else mybir.dt.float32
NPDT = ml_dtypes.bfloat16 if USE_BF16 else np.float32
DT_OUT = mybir.dt.float32

_CACHE = {}


def _build_body(nc, tpool, wpool, opool, pspool, w_d, te_d, to_d, out_d):
    # te/to + out DMAs ride the scalar (ACT) HWDGE ring so the sync (SP)
    # ring carries only the big weight stream (no head-of-line blocking).
    te = tpool.tile([128, PAIRS + 1, 16], DT)
    to = tpool.tile([128, PAIRS, 16], DT)
    nc.scalar.dma_start(te[:], te_d[:])
    nc.scalar.dma_start(to[:], to_d[:])

    for h in range(PAIRS // W_BLK):  # weight blocks of W_BLK pairs
        wb = wpool.tile([128, W_BLK, K, F], DT)
        nc.sync.dma_start(wb[:], w_d[:, ds(h * W_BLK, W_BLK)])
        # 4 groups of 8 pairs; group q runs in PE column strip q and
        # accumulates into its own PSUM bank at partition offset 32*q.
        accs = [pspool.tile([128, 512], DT_OUT, name=f"acc{q}", tag=f"acc{q}")
                for q in range(4)]
        for j in range(8):
            for q in range(4):
                i = h * W_BLK + q * 8 + j   # global pair
                jj = q * 8 + j              # pair in W block
                o_ap = accs[q][ds(32 * q, 16), ts(j, 64)]
                tp = (0, 32 * q)
                nc.tensor.matmul(o_ap, te[:, i, :], wb[:, jj, 0, :],
                                 start=True, stop=False, tile_position=tp)
                nc.tensor.matmul(o_ap, to[:, i, :], wb[:, jj, 1, :],
                                 start=False, stop=False, tile_position=tp)
                nc.tensor.matmul(o_ap, te[:, i + 1, :], wb[:, jj, 2, :],
                                 start=False, stop=True, tile_position=tp)
        for q in range(4):
            g = h * 4 + q  # global group (0..31)
            ob = opool.tile([16, 512], DT_OUT)
            nc.vector.tensor_copy(ob[:], accs[q][ds(32 * q, 16), :])
            nc.scalar.dma_start(out_d[g], ob[:])


def _build_nc(n_iters=None):
    """n_iters=None: straight-line kernel (graded path).
    n_iters=N: body wrapped in a HW For_i loop, for timing-slope runs."""
    nc = bacc.Bacc("TRN2", target_bir_lowering=False, debug=False)

    w_d = nc.declare_dram_parameter("w", [128, PAIRS, K, F], DT, isOutput=False)
    te_d = nc.declare_dram_parameter("te", [128, PAIRS + 1, 16], DT, isOutput=False)
    to_d = nc.declare_dram_parameter("to", [128, PAIRS, 16], DT, isOutput=False)
    # out[g, m, j*64+f]: g = group of 8 pairs (32 total), m = phase*8 + b.
    out_d = nc.declare_dram_parameter("out", [PAIRS // 8, 16, 512], DT_OUT,
                                      isOutput=True)

    with tile.TileContext(nc) as tc:
        with (
            tc.tile_pool(name="tpool", bufs=1) as tpool,
            tc.tile_pool(name="wpool", bufs=5) as wpool,
            tc.tile_pool(name="opool", bufs=8) as opool,
            # 4 acc tags (one per PE strip) x 2 bufs = all 8 PSUM banks
            tc.tile_pool(name="pspool", bufs=2, space=bass.MemorySpace.PSUM) as pspool,
        ):
            if n_iters is None:
                _build_body(nc, tpool, wpool, opool, pspool, w_d, te_d, to_d, out_d)
            else:
                with tc.For_i(0, n_iters, 1):
                    _build_body(nc, tpool, wpool, opool, pspool,
                                w_d, te_d, to_d, out_d)

    nc.compile()
    return nc


def _prep_inputs(x, kernel):
    """Host-side rearrangement into per-core DMA-friendly layouts."""
    xp = np.zeros((B, L + 4, C), np.float32)
    xp[:, :L] = x
    kp = np.zeros((N_CORES * P_CORE, K, C, F), np.float32)
    kp[:L_OUT] = kernel
    in_maps = []
    for m in range(N_CORES):
        l0 = P_CORE * m
        xs = xp[:, l0:l0 + 2 * PAIRS + 2, :]
        ev = np.ascontiguousarray(xs[:, 0::2].transpose(2, 1, 0))  # (64, 257, 8) j=2i
        od = np.ascontiguousarray(xs[:, 1::2].transpose(2, 1, 0))  # (64, 257, 8) j=2i+1
        TE = np.zeros((128, PAIRS + 1, 16), NPDT)
        TE[:64, :, 0:8] = ev
        TE[64:, :, 8:16] = od
        TO = np.zeros((128, PAIRS, 16), NPDT)
        TO[:64, :, 0:8] = od[:, :PAIRS]
        TO[64:, :, 8:16] = ev[:, 1:PAIRS + 1]
        W = (kp[l0:l0 + P_CORE]
             .reshape(PAIRS, 2, K, C, F)
             .transpose(1, 3, 0, 2, 4)
             .reshape(128, PAIRS, K, F)
             .astype(NPDT))
        in_maps.append({"w": W, "te": TE, "to": TO})
    return in_maps


def _unpack_out(res):
    """(32,16,512) per core -> (B, P_CORE, F).  l_local = 16g + 2j + phase."""
    return (res.reshape(32, 2, 8, 8, 64)          # [g, phase, b, j, f]
            .transpose(2, 0, 3, 1, 4)              # [b, g, j, phase, f]
            .reshape(B, P_CORE, F))


def kernel(x, kernel, bias):
    x = np.asarray(x, dtype=np.float32)
    kern = np.asarray(kernel, dtype=np.float32)
    bias = np.asarray(bias, dtype=np.float32)

    if "nc" not in _CACHE:
        _CACHE["nc"] = _build_nc()
    nc = _CACHE["nc"]

    in_maps = _prep_inputs(x, kern)
    results = run_bass_kernel_spmd(nc, in_maps, list(range(N_CORES))).results

    parts = [_unpack_out(results[m]["out"]) for m in range(N_CORES)]
    out = np.concatenate(parts, axis=1)[:, :L_OUT]
    return (out + bias[None]).astype(np.float32)


# revision 16
# speedup vs baseline: 70703.5237x; 1.0685x over previous
"""LocallyConnected1D (B=8, L=4096, C=64, K=3, F=64) on 8 TRN2 NeuronCores.

out[b, l, f] = sum_{k,c} x[b, l+k, c] * kernel[l, k, c, f] + bias[l, f]

Strategy (spatial sharding, 512 output positions per core):
  - For each pair of adjacent output positions (l0+2i, l0+2i+1) build a
    block-diagonal stationary tile lhsT (128 x 16): partitions = 2 phases x 64
    channels, columns = 2 phases x 8 batch.  Streaming operand = the pair's
    per-position weights (128 x 64).  Three PSUM-accumulated matmuls per pair
    (one per tap k, using x-pair tiles shifted by k) produce out (16, 64).
  - Groups of 8 pairs are dispatched to 4 independent 32-column strips of the
    PE array (tile_position), each strip accumulating into its own PSUM bank,
    so up to 4 matmuls run concurrently in the array.
  - Per block of 32 pairs, the weights AND the x-pair tiles are packed into
    ONE contiguous DRAM block -> a single dense DMA per block at full HBM
    bandwidth, minimal startup serialization.
  - Compute in bf16 (PSUM accumulation in f32); bias added on host.
"""

import numpy as np
import ml_dtypes

import concourse.bass as bass
import concourse.mybir as mybir
import concourse.tile as tile
from concourse import bacc
from concourse.bass import ds, ts
from concourse.bass_utils import run_bass_kernel_spmd

B, L, C, K, F = 8, 4096, 64, 3, 64
L_OUT = (L - K) + 1  # 4094
N_CORES = 8
P_CORE = 512          # output positions per core (last core: 510 real + 2 pad)
PAIRS = P_CORE // 2   # 256
W_BLK = 32            # pairs per DMA block
N_BLK = PAIRS // W_BLK

USE_BF16 = True
DT = mybir.dt.bfloat16 if USE_BF16 else mybir.dt.float32
NPDT = ml_dtypes.bfloat16 if USE_BF16 else np.float32
DT_OUT = mybir.dt.float32

# per-block column layout (per partition): weights | te tiles | to tiles
W_COLS = W_BLK * K * F           # 6144
TE_COLS = (W_BLK + 1) * 16       # 528  (one extra pair tile for tap k=2)
TO_COLS = W_BLK * 16             # 512
BLK_COLS = W_COLS + TE_COLS + TO_COLS  # 7184

_CACHE = {}


def _build_body(nc, wpool, opool, pspool, blk_d, out_d):
    for h in range(N_BLK):
        blk = wpool.tile([128, BLK_COLS], DT, name="blk", tag="blk")
        nc.sync.dma_start(blk[:], blk_d[h])
        accs = [pspool.tile([128, 512], DT_OUT, name=f"acc{q}", tag=f"acc{q}")
                for q in range(4)]

        def te_ap(i):   # pair tile for even-start pair i (block-local +1 ok)
            return blk[:, ds(W_COLS + (i - h * W_BLK) * 16, 16)]

        def to_ap(i):
            return blk[:, ds(W_COLS + TE_COLS + (i - h * W_BLK) * 16, 16)]

        def w_ap(jj, k):
            return blk[:, ds((jj * K + k) * F, F)]

        for j in range(8):
            for q in range(4):
                i = h * W_BLK + q * 8 + j   # global pair
                jj = q * 8 + j              # pair in block
                o_ap = accs[q][ds(32 * q, 16), ts(j, 64)]
                tp = (0, 32 * q)
                nc.tensor.matmul(o_ap, te_ap(i), w_ap(jj, 0),
                                 start=True, stop=False, tile_position=tp)
                nc.tensor.matmul(o_ap, to_ap(i), w_ap(jj, 1),
                                 start=False, stop=False, tile_position=tp)
                nc.tensor.matmul(o_ap, te_ap(i + 1), w_ap(jj, 2),
                                 start=False, stop=True, tile_position=tp)
        for q in range(4):
            g = h * 4 + q  # global group (0..31)
            ob = opool.tile([16, 512], DT_OUT, name="ob", tag="ob")
            nc.vector.tensor_copy(ob[:], accs[q][ds(32 * q, 16), :])
            nc.scalar.dma_start(out_d[g], ob[:])


def _build_nc(n_iters=None):
    """n_iters=None: straight-line kernel (graded path).
    n_iters=N: body wrapped in a HW For_i loop, for timing-slope runs."""
    nc = bacc.Bacc("TRN2", target_bir_lowering=False, debug=False)

    blk_d = nc.declare_dram_parameter("blk", [N_BLK, 128, BLK_COLS], DT,
                                      isOutput=False)
    # out[g, m, j*64+f]: g = group of 8 pairs (32 total), m = phase*8 + b.
    out_d = nc.declare_dram_parameter("out", [PAIRS // 8, 16, 512], DT_OUT,
                                      isOutput=True)

    with tile.TileContext(nc) as tc:
        with (
            tc.tile_pool(name="wpool", bufs=5) as wpool,
            tc.tile_pool(name="opool", bufs=8) as opool,
            # 4 acc tags (one per PE strip) x 2 bufs = all 8 PSUM banks
            tc.tile_pool(name="pspool", bufs=2, space=bass.MemorySpace.PSUM) as pspool,
        ):
            if n_iters is None:
                _build_body(nc, wpool, opool, pspool, blk_d, out_d)
            else:
                with tc.For_i(0, n_iters, 1):
                    _build_body(nc, wpool, opool, pspool, blk_d, out_d)

    nc.compile()
    return nc


def _prep_inputs(x, kernel):
    """Host-side rearrangement into per-core fused block layouts."""
    xp = np.zeros((B, L + 4, C), np.float32)
    xp[:, :L] = x
    kp = np.zeros((N_CORES * P_CORE, K, C, F), np.float32)
    kp[:L_OUT] = kernel
    in_maps = []
    for m in range(N_CORES):
        l0 = P_CORE * m
        xs = xp[:, l0:l0 + 2 * PAIRS + 2, :]
        ev = xs[:, 0::2].transpose(2, 1, 0)  # (64, 257, 8)  j = 2i
        od = xs[:, 1::2].transpose(2, 1, 0)  # (64, 257, 8)  j = 2i+1
        # TE[i]: pair (2i, 2i+1); TO[i]: pair (2i+1, 2i+2); block-diag (128,16)
        TE = np.zeros((128, PAIRS + 1, 16), np.float32)
        TE[:64, :, 0:8] = ev
        TE[64:, :, 8:16] = od
        TO = np.zeros((128, PAIRS, 16), np.float32)
        TO[:64, :, 0:8] = od[:, :PAIRS]
        TO[64:, :, 8:16] = ev[:, 1:PAIRS + 1]
        W = (kp[l0:l0 + P_CORE]
             .reshape(PAIRS, 2, K, C, F)
             .transpose(1, 3, 0, 2, 4)
             .reshape(128, PAIRS, K, F))  # [pc, pair, k, f]
        blk = np.empty((N_BLK, 128, BLK_COLS), np.float32)
        for h in range(N_BLK):
            s = h * W_BLK
            blk[h, :, :W_COLS] = W[:, s:s + W_BLK].reshape(128, W_COLS)
            blk[h, :, W_COLS:W_COLS + TE_COLS] = (
                TE[:, s:s + W_BLK + 1].reshape(128, TE_COLS))
            blk[h, :, W_COLS + TE_COLS:] = (
                TO[:, s:s + W_BLK].reshape(128, TO_COLS))
        in_maps.append({"blk": blk.astype(NPDT)})
    return in_maps


def _unpack_out(res):
    """(32,16,512) per core -> (B, P_CORE, F).  l_local = 16g + 2j + phase."""
    return (res.reshape(32, 2, 8, 8, 64)          # [g, phase, b, j, f]
            .transpose(2, 0, 3, 1, 4)              # [b, g, j, phase, f]
            .reshape(B, P_CORE, F))


def kernel(x, kernel, bias):
    x = np.asarray(x, dtype=np.float32)
    kern = np.asarray(kernel, dtype=np.float32)
    bias = np.asarray(bias, dtype=np.float32)

    if "nc" not in _CACHE:
        _CACHE["nc"] = _build_nc()
    nc = _CACHE["nc"]

    in_maps = _prep_inputs(x, kern)
    results = run_bass_kernel_spmd(nc, in_maps, list(range(N_CORES))).results

    parts = [_unpack_out(results[m]["out"]) for m in range(N_CORES)]
    out = np.concatenate(parts, axis=1)[:, :L_OUT]
    return (out + bias[None]).astype(np.float32)


# revision 17
# speedup vs baseline: 71932.5866x; 1.0174x over previous
"""LocallyConnected1D (B=8, L=4096, C=64, K=3, F=64) on 8 TRN2 NeuronCores.

out[b, l, f] = sum_{k,c} x[b, l+k, c] * kernel[l, k, c, f] + bias[l, f]

Strategy (spatial sharding, 512 output positions per core):
  - For each pair of adjacent output positions (l0+2i, l0+2i+1) build a
    block-diagonal stationary tile lhsT (128 x 16): partitions = 2 phases x 64
    channels, columns = 2 phases x 8 batch.  Streaming operand = the pair's
    per-position weights (128 x 64).  Three PSUM-accumulated matmuls per pair
    (one per tap k, using x-pair tiles shifted by k) produce out (16, 64).
  - Groups of 8 pairs are dispatched to 4 independent 32-column strips of the
    PE array (tile_position), each strip accumulating into its own PSUM bank,
    so up to 4 matmuls run concurrently in the array.
  - Weights AND x-pair tiles are packed into one contiguous DRAM blob per
    block -> dense DMAs at full HBM bandwidth.  First blocks are small so the
    PE starts early; per-block outputs go out in a single DMA.
  - Compute in bf16 (PSUM accumulation in f32); bias added on host.
"""

import numpy as np
import ml_dtypes

import concourse.bass as bass
import concourse.mybir as mybir
import concourse.tile as tile
from concourse import bacc
from concourse.bass import ds, ts
from concourse.bass_utils import run_bass_kernel_spmd

B, L, C, K, F = 8, 4096, 64, 3, 64
L_OUT = (L - K) + 1  # 4094
N_CORES = 8
P_CORE = 512          # output positions per core (last core: 510 real + 2 pad)
PAIRS = P_CORE // 2   # 256

# pairs per DMA block; small first blocks let the PE start early
BLOCKS = [8, 8, 16] + [32] * 7
assert sum(BLOCKS) == PAIRS and all(b % 8 == 0 for b in BLOCKS)

USE_BF16 = True
DT = mybir.dt.bfloat16 if USE_BF16 else mybir.dt.float32
NPDT = ml_dtypes.bfloat16 if USE_BF16 else np.float32
DT_OUT = mybir.dt.float32

# per-block columns (per partition): weights | te tiles | to tiles
def _blk_cols(n):
    return n * K * F + (n + 1) * 16 + n * 16

BLK_OFF = np.cumsum([0] + [_blk_cols(n) for n in BLOCKS]).tolist()
TOT_COLS = BLK_OFF[-1]

_CACHE = {}


def _build_body(nc, wpool, opool, pspool, blk_d, out_d):
    s = 0  # first pair of current block
    for h, n in enumerate(BLOCKS):
        cols = _blk_cols(n)
        blk = wpool.tile([128, cols], DT, name="blk", tag="blk",
                         padded_shape=[128, _blk_cols(max(BLOCKS))])
        nc.sync.dma_start(blk[:], blk_d[:, ds(BLK_OFF[h], cols)])
        w_cols = n * K * F
        te_cols = (n + 1) * 16
        ngroups = n // 8
        accs = [pspool.tile([128, 512], DT_OUT, name=f"acc{q}", tag=f"acc{q}")
                for q in range(ngroups)]

        def te_ap(i):   # block-diag tile for even-start pair i
            return blk[:, ds(w_cols + (i - s) * 16, 16)]

        def to_ap(i):   # odd-start pair i
            return blk[:, ds(w_cols + te_cols + (i - s) * 16, 16)]

        def w_ap(jj, k):
            return blk[:, ds((jj * K + k) * F, F)]

        for j in range(8):
            for q in range(ngroups):
                i = s + q * 8 + j   # global pair
                jj = q * 8 + j      # pair in block
                o_ap = accs[q][ds(32 * q, 16), ts(j, 64)]
                tp = (0, 32 * q)
                nc.tensor.matmul(o_ap, te_ap(i), w_ap(jj, 0),
                                 start=True, stop=False, tile_position=tp)
                nc.tensor.matmul(o_ap, to_ap(i), w_ap(jj, 1),
                                 start=False, stop=False, tile_position=tp)
                nc.tensor.matmul(o_ap, te_ap(i + 1), w_ap(jj, 2),
                                 start=False, stop=True, tile_position=tp)
        ob = opool.tile([16, ngroups * 512], DT_OUT, name="ob", tag="ob",
                        padded_shape=[16, 4 * 512])
        for q in range(ngroups):
            nc.vector.tensor_copy(ob[:, ds(q * 512, 512)],
                                  accs[q][ds(32 * q, 16), :])
        g0 = s // 8  # first global group of this block
        nc.scalar.dma_start(out_d[:, ds(g0 * 512, ngroups * 512)], ob[:])
        s += n


def _build_nc(n_iters=None):
    """n_iters=None: straight-line kernel (graded path).
    n_iters=N: body wrapped in a HW For_i loop, for timing-slope runs."""
    nc = bacc.Bacc("TRN2", target_bir_lowering=False, debug=False)

    blk_d = nc.declare_dram_parameter("blk", [128, TOT_COLS], DT, isOutput=False)
    # out[m, g*512 + j*64 + f]: g = group of 8 pairs, m = phase*8 + b.
    out_d = nc.declare_dram_parameter("out", [16, (PAIRS // 8) * 512], DT_OUT,
                                      isOutput=True)

    with tile.TileContext(nc) as tc:
        with (
            tc.tile_pool(name="wpool", bufs=6) as wpool,
            tc.tile_pool(name="opool", bufs=4) as opool,
            # 4 acc tags (one per PE strip) x 2 bufs = all 8 PSUM banks
            tc.tile_pool(name="pspool", bufs=2, space=bass.MemorySpace.PSUM) as pspool,
        ):
            if n_iters is None:
                _build_body(nc, wpool, opool, pspool, blk_d, out_d)
            else:
                with tc.For_i(0, n_iters, 1):
                    _build_body(nc, wpool, opool, pspool, blk_d, out_d)

    nc.compile()
    return nc


def _prep_inputs(x, kernel):
    """Host-side rearrangement into per-core fused block layouts."""
    xp = np.zeros((B, L + 4, C), np.float32)
    xp[:, :L] = x
    kp = np.zeros((N_CORES * P_CORE, K, C, F), np.float32)
    kp[:L_OUT] = kernel
    in_maps = []
    for m in range(N_CORES):
        l0 = P_CORE * m
        xs = xp[:, l0:l0 + 2 * PAIRS + 2, :]
        ev = xs[:, 0::2].transpose(2, 1, 0)  # (64, 257, 8)  j = 2i
        od = xs[:, 1::2].transpose(2, 1, 0)  # (64, 257, 8)  j = 2i+1
        # TE[i]: pair (2i, 2i+1); TO[i]: pair (2i+1, 2i+2); block-diag (128,16)
        TE = np.zeros((128, PAIRS + 1, 16), np.float32)
        TE[:64, :, 0:8] = ev
        TE[64:, :, 8:16] = od
        TO = np.zeros((128, PAIRS, 16), np.float32)
        TO[:64, :, 0:8] = od[:, :PAIRS]
        TO[64:, :, 8:16] = ev[:, 1:PAIRS + 1]
        W = (kp[l0:l0 + P_CORE]
             .reshape(PAIRS, 2, K, C, F)
             .transpose(1, 3, 0, 2, 4)
             .reshape(128, PAIRS, K, F))  # [pc, pair, k, f]
        blk = np.empty((128, TOT_COLS), np.float32)
        s = 0
        for h, n in enumerate(BLOCKS):
            o = BLK_OFF[h]
            w_cols = n * K * F
            blk[:, o:o + w_cols] = W[:, s:s + n].reshape(128, w_cols)
            blk[:, o + w_cols:o + w_cols + (n + 1) * 16] = (
                TE[:, s:s + n + 1].reshape(128, (n + 1) * 16))
            blk[:, o + w_cols + (n + 1) * 16:o + _blk_cols(n)] = (
                TO[:, s:s + n].reshape(128, n * 16))
            s += n
        in_maps.append({"blk": blk.astype(NPDT)})
    return in_maps


def _unpack_out(res):
    """(16, 32*512) per core -> (B, P_CORE, F).  l_local = 16g + 2j + phase."""
    return (res.reshape(2, 8, 32, 8, 64)          # [phase, b, g, j, f]
            .transpose(1, 2, 3, 0, 4)              # [b, g, j, phase, f]
            .reshape(B, P_CORE, F))


def kernel(x, kernel, bias):
    x = np.asarray(x, dtype=np.float32)
    kern = np.asarray(kernel, dtype=np.float32)
    bias = np.asarray(bias, dtype=np.float32)

    if "nc" not in _CACHE:
        _CACHE["nc"] = _build_nc()
    nc = _CACHE["nc"]

    in_maps = _prep_inputs(x, kern)
    results = run_bass_kernel_spmd(nc, in_maps, list(range(N_CORES))).results

    parts = [_unpack_out(results[m]["out"]) for m in range(N_CORES)]
    out = np.concatenate(parts, axis=1)[:, :L_OUT]
    return (out + bias[None]).astype(np.float32)


# revision 18
# speedup vs baseline: 75086.9690x; 1.0439x over previous
"""LocallyConnected1D (B=8, L=4096, C=64, K=3, F=64) on 8 TRN2 NeuronCores.

out[b, l, f] = sum_{k,c} x[b, l+k, c] * kernel[l, k, c, f] + bias[l, f]

Strategy (spatial sharding, 512 output positions per core):
  - For each pair of adjacent output positions (l0+2i, l0+2i+1) build a
    block-diagonal stationary tile lhsT (128 x 16): partitions = 2 phases x 64
    channels, columns = 2 phases x 8 batch.  Streaming operand = the pair's
    per-position weights (128 x 64).  Three PSUM-accumulated matmuls per pair
    (one per tap k, using x-pair tiles shifted by k) produce out (16, 64).
  - Groups of 8 pairs are dispatched to 4 independent 32-column strips of the
    PE array (tile_position), each strip accumulating into its own PSUM bank,
    so up to 4 matmuls run concurrently in the array.
  - Weights AND x-pair tiles are packed into one contiguous DRAM blob per
    block -> dense DMAs at full HBM bandwidth.  First blocks are small so the
    PE starts early; per-block outputs go out in a single DMA.
  - Compute in bf16 (PSUM accumulation in f32); bias added on host.
"""

import numpy as np
import ml_dtypes

import concourse.bass as bass
import concourse.mybir as mybir
import concourse.tile as tile
from concourse import bacc
from concourse.bass import ds, ts
from concourse.bass_utils import run_bass_kernel_spmd

B, L, C, K, F = 8, 4096, 64, 3, 64
L_OUT = (L - K) + 1  # 4094
N_CORES = 8
P_CORE = 512          # output positions per core (last core: 510 real + 2 pad)
PAIRS = P_CORE // 2   # 256

# pairs per DMA block; small first blocks let the PE start early
BLOCKS = [8, 8, 16] + [32] * 6 + [16, 8, 8]
assert sum(BLOCKS) == PAIRS and all(b % 8 == 0 for b in BLOCKS)

USE_BF16 = True
DT = mybir.dt.bfloat16 if USE_BF16 else mybir.dt.float32
NPDT = ml_dtypes.bfloat16 if USE_BF16 else np.float32
DT_OUT = mybir.dt.float32

# per-block columns (per partition): weights | te tiles | to tiles
def _blk_cols(n):
    return n * K * F + (n + 1) * 16 + n * 16

BLK_OFF = np.cumsum([0] + [_blk_cols(n) for n in BLOCKS]).tolist()
TOT_COLS = BLK_OFF[-1]

_CACHE = {}


def _build_body(nc, wpool, opool, pspool, blk_d, out_d):
    s = 0  # first pair of current block
    for h, n in enumerate(BLOCKS):
        cols = _blk_cols(n)
        blk = wpool.tile([128, cols], DT, name="blk", tag="blk",
                         padded_shape=[128, _blk_cols(max(BLOCKS))])
        nc.sync.dma_start(blk[:], blk_d[:, ds(BLK_OFF[h], cols)])
        w_cols = n * K * F
        te_cols = (n + 1) * 16
        ngroups = n // 8
        accs = [pspool.tile([128, 512], DT_OUT, name=f"acc{q}", tag=f"acc{q}")
                for q in range(ngroups)]

        def te_ap(i):   # block-diag tile for even-start pair i
            return blk[:, ds(w_cols + (i - s) * 16, 16)]

        def to_ap(i):   # odd-start pair i
            return blk[:, ds(w_cols + te_cols + (i - s) * 16, 16)]

        def w_ap(jj, k):
            return blk[:, ds((jj * K + k) * F, F)]

        for j in range(8):
            for q in range(ngroups):
                i = s + q * 8 + j   # global pair
                jj = q * 8 + j      # pair in block
                o_ap = accs[q][ds(32 * q, 16), ts(j, 64)]
                tp = (0, 32 * q)
                nc.tensor.matmul(o_ap, te_ap(i), w_ap(jj, 0),
                                 start=True, stop=False, tile_position=tp)
                nc.tensor.matmul(o_ap, to_ap(i), w_ap(jj, 1),
                                 start=False, stop=False, tile_position=tp)
                nc.tensor.matmul(o_ap, te_ap(i + 1), w_ap(jj, 2),
                                 start=False, stop=True, tile_position=tp)
        ob = opool.tile([16, ngroups * 512], DT_OUT, name="ob", tag="ob",
                        padded_shape=[16, 4 * 512])
        for q in range(ngroups):
            nc.vector.tensor_copy(ob[:, ds(q * 512, 512)],
                                  accs[q][ds(32 * q, 16), :])
        g0 = s // 8  # first global group of this block
        nc.scalar.dma_start(out_d[:, ds(g0 * 512, ngroups * 512)], ob[:])
        s += n


def _build_nc(n_iters=None):
    """n_iters=None: straight-line kernel (graded path).
    n_iters=N: body wrapped in a HW For_i loop, for timing-slope runs."""
    nc = bacc.Bacc("TRN2", target_bir_lowering=False, debug=False)

    blk_d = nc.declare_dram_parameter("blk", [128, TOT_COLS], DT, isOutput=False)
    # out[m, g*512 + j*64 + f]: g = group of 8 pairs, m = phase*8 + b.
    out_d = nc.declare_dram_parameter("out", [16, (PAIRS // 8) * 512], DT_OUT,
                                      isOutput=True)

    with tile.TileContext(nc) as tc:
        with (
            tc.tile_pool(name="wpool", bufs=8) as wpool,
            tc.tile_pool(name="opool", bufs=4) as opool,
            # 4 acc tags (one per PE strip) x 2 bufs = all 8 PSUM banks
            tc.tile_pool(name="pspool", bufs=2, space=bass.MemorySpace.PSUM) as pspool,
        ):
            if n_iters is None:
                _build_body(nc, wpool, opool, pspool, blk_d, out_d)
            else:
                with tc.For_i(0, n_iters, 1):
                    _build_body(nc, wpool, opool, pspool, blk_d, out_d)

    nc.compile()
    return nc


def _prep_inputs(x, kernel):
    """Host-side rearrangement into per-core fused block layouts."""
    xp = np.zeros((B, L + 4, C), np.float32)
    xp[:, :L] = x
    kp = np.zeros((N_CORES * P_CORE, K, C, F), np.float32)
    kp[:L_OUT] = kernel
    in_maps = []
    for m in range(N_CORES):
        l0 = P_CORE * m
        xs = xp[:, l0:l0 + 2 * PAIRS + 2, :]
        ev = xs[:, 0::2].transpose(2, 1, 0)  # (64, 257, 8)  j = 2i
        od = xs[:, 1::2].transpose(2, 1, 0)  # (64, 257, 8)  j = 2i+1
        # TE[i]: pair (2i, 2i+1); TO[i]: pair (2i+1, 2i+2); block-diag (128,16)
        TE = np.zeros((128, PAIRS + 1, 16), np.float32)
        TE[:64, :, 0:8] = ev
        TE[64:, :, 8:16] = od
        TO = np.zeros((128, PAIRS, 16), np.float32)
        TO[:64, :, 0:8] = od[:, :PAIRS]
        TO[64:, :, 8:16] = ev[:, 1:PAIRS + 1]
        W = (kp[l0:l0 + P_CORE]
             .reshape(PAIRS, 2, K, C, F)
             .transpose(1, 3, 0, 2, 4)
             .reshape(128, PAIRS, K, F))  # [pc, pair, k, f]
        blk = np.empty((128, TOT_COLS), np.float32)
        s = 0
        for h, n in enumerate(BLOCKS):
            o = BLK_OFF[h]
            w_cols = n * K * F
            blk[:, o:o + w_cols] = W[:, s:s + n].reshape(128, w_cols)
            blk[:, o + w_cols:o + w_cols + (n + 1) * 16] = (
                TE[:, s:s + n + 1].reshape(128, (n + 1) * 16))
            blk[:, o + w_cols + (n + 1) * 16:o + _blk_cols(n)] = (
                TO[:, s:s + n].reshape(128, n * 16))
            s += n
        in_maps.append({"blk": blk.astype(NPDT)})
    return in_maps


def _unpack_out(res):
    """(16, 32*512) per core -> (B, P_CORE, F).  l_local = 16g + 2j + phase."""
    return (res.reshape(2, 8, 32, 8, 64)          # [phase, b, g, j, f]
            .transpose(1, 2, 3, 0, 4)              # [b, g, j, phase, f]
            .reshape(B, P_CORE, F))


def kernel(x, kernel, bias):
    x = np.asarray(x, dtype=np.float32)
    kern = np.asarray(kernel, dtype=np.float32)
    bias = np.asarray(bias, dtype=np.float32)

    if "nc" not in _CACHE:
        _CACHE["nc"] = _build_nc()
    nc = _CACHE["nc"]

    in_maps = _prep_inputs(x, kern)
    results = run_bass_kernel_spmd(nc, in_maps, list(range(N_CORES))).results

    parts = [_unpack_out(results[m]["out"]) for m in range(N_CORES)]
    out = np.concatenate(parts, axis=1)[:, :L_OUT]
    return (out + bias[None]).astype(np.float32)


# revision 19
# speedup vs baseline: 77439.3195x; 1.0313x over previous
"""LocallyConnected1D (B=8, L=4096, C=64, K=3, F=64) on 8 TRN2 NeuronCores.

out[b, l, f] = sum_{k,c} x[b, l+k, c] * kernel[l, k, c, f] + bias[l, f]

Strategy (spatial sharding, 512 output positions per core):
  - For each pair of adjacent output positions (l0+2i, l0+2i+1) build a
    block-diagonal stationary tile lhsT (128 x 16): partitions = 2 phases x 64
    channels, columns = 2 phases x 8 batch.  Streaming operand = the pair's
    per-position weights (128 x 64).  Three PSUM-accumulated matmuls per pair
    (one per tap k, using x-pair tiles shifted by k) produce out (16, 64).
  - Groups of 8 pairs are dispatched to 4 independent 32-column strips of the
    PE array (tile_position), each strip accumulating into its own PSUM bank,
    so up to 4 matmuls run concurrently in the array.
  - Weights AND x-pair tiles are packed into one contiguous DRAM blob per
    block -> dense DMAs at full HBM bandwidth.  First blocks are small so the
    PE starts early; per-block outputs go out in a single DMA.
  - Compute in bf16 (PSUM accumulation in f32); bias added on host.
"""

import numpy as np
import ml_dtypes

import concourse.bass as bass
import concourse.mybir as mybir
import concourse.tile as tile
from concourse import bacc
from concourse.bass import ds, ts
from concourse.bass_utils import run_bass_kernel_spmd

B, L, C, K, F = 8, 4096, 64, 3, 64
L_OUT = (L - K) + 1  # 4094
N_CORES = 8
P_CORE = 512          # output positions per core (last core: 510 real + 2 pad)
PAIRS = P_CORE // 2   # 256

# pairs per DMA block; small first blocks let the PE start early
BLOCKS = [8, 8, 16] + [32] * 6 + [16, 8, 8]
assert sum(BLOCKS) == PAIRS and all(b % 8 == 0 for b in BLOCKS)

USE_BF16 = True
DT = mybir.dt.bfloat16 if USE_BF16 else mybir.dt.float32
NPDT = ml_dtypes.bfloat16 if USE_BF16 else np.float32
DT_OUT = mybir.dt.float32

# per-block columns (per partition): weights | te tiles | to tiles
def _blk_cols(n):
    return n * K * F + (n + 1) * 16 + n * 16

BLK_OFF = np.cumsum([0] + [_blk_cols(n) for n in BLOCKS]).tolist()
TOT_COLS = BLK_OFF[-1]

_CACHE = {}


def _build_body(nc, wpool, opool, pspool, blk_d, out_d):
    s = 0  # first pair of current block
    for h, n in enumerate(BLOCKS):
        cols = _blk_cols(n)
        blk = wpool.tile([128, cols], DT, name="blk", tag="blk",
                         padded_shape=[128, _blk_cols(max(BLOCKS))])
        nc.sync.dma_start(blk[:], blk_d[:, ds(BLK_OFF[h], cols)])
        w_cols = n * K * F
        te_cols = (n + 1) * 16
        ngroups = n // 8
        accs = [pspool.tile([128, 512], DT_OUT, name=f"acc{q}", tag=f"acc{q}")
                for q in range(ngroups)]

        def te_ap(i):   # block-diag tile for even-start pair i
            return blk[:, ds(w_cols + (i - s) * 16, 16)]

        def to_ap(i):   # odd-start pair i
            return blk[:, ds(w_cols + te_cols + (i - s) * 16, 16)]

        def w_ap(jj, k):
            return blk[:, ds((jj * K + k) * F, F)]

        for j in range(8):
            for q in range(ngroups):
                i = s + q * 8 + j   # global pair
                jj = q * 8 + j      # pair in block
                o_ap = accs[q][ds(32 * q, 16), ts(j, 64)]
                tp = (0, 32 * q)
                nc.tensor.matmul(o_ap, te_ap(i), w_ap(jj, 0),
                                 start=True, stop=False, tile_position=tp)
                nc.tensor.matmul(o_ap, to_ap(i), w_ap(jj, 1),
                                 start=False, stop=False, tile_position=tp)
                nc.tensor.matmul(o_ap, te_ap(i + 1), w_ap(jj, 2),
                                 start=False, stop=True, tile_position=tp)
        ob = opool.tile([16, ngroups * 512], DT_OUT, name="ob", tag="ob",
                        padded_shape=[16, 4 * 512])
        for q in range(ngroups):
            nc.vector.tensor_copy(ob[:, ds(q * 512, 512)],
                                  accs[q][ds(32 * q, 16), :])
        g0 = s // 8  # first global group of this block
        nc.scalar.dma_start(out_d[:, ds(g0 * 512, ngroups * 512)], ob[:])
        s += n


def _build_nc(n_iters=None):
    """n_iters=None: straight-line kernel (graded path).
    n_iters=N: body wrapped in a HW For_i loop, for timing-slope runs."""
    nc = bacc.Bacc("TRN2", target_bir_lowering=False, debug=False)

    blk_d = nc.declare_dram_parameter("blk", [128, TOT_COLS], DT, isOutput=False)
    # out[m, g*512 + j*64 + f]: g = group of 8 pairs, m = phase*8 + b.
    out_d = nc.declare_dram_parameter("out", [16, (PAIRS // 8) * 512], DT_OUT,
                                      isOutput=True)

    with tile.TileContext(nc) as tc:
        with (
            tc.tile_pool(name="wpool", bufs=8) as wpool,
            tc.tile_pool(name="opool", bufs=8) as opool,
            # 4 acc tags (one per PE strip) x 2 bufs = all 8 PSUM banks
            tc.tile_pool(name="pspool", bufs=2, space=bass.MemorySpace.PSUM) as pspool,
        ):
            if n_iters is None:
                _build_body(nc, wpool, opool, pspool, blk_d, out_d)
            else:
                with tc.For_i(0, n_iters, 1):
                    _build_body(nc, wpool, opool, pspool, blk_d, out_d)

    nc.compile()
    return nc


def _prep_inputs(x, kernel):
    """Host-side rearrangement into per-core fused block layouts."""
    xp = np.zeros((B, L + 4, C), np.float32)
    xp[:, :L] = x
    kp = np.zeros((N_CORES * P_CORE, K, C, F), np.float32)
    kp[:L_OUT] = kernel
    in_maps = []
    for m in range(N_CORES):
        l0 = P_CORE * m
        xs = xp[:, l0:l0 + 2 * PAIRS + 2, :]
        ev = xs[:, 0::2].transpose(2, 1, 0)  # (64, 257, 8)  j = 2i
        od = xs[:, 1::2].transpose(2, 1, 0)  # (64, 257, 8)  j = 2i+1
        # TE[i]: pair (2i, 2i+1); TO[i]: pair (2i+1, 2i+2); block-diag (128,16)
        TE = np.zeros((128, PAIRS + 1, 16), np.float32)
        TE[:64, :, 0:8] = ev
        TE[64:, :, 8:16] = od
        TO = np.zeros((128, PAIRS, 16), np.float32)
        TO[:64, :, 0:8] = od[:, :PAIRS]
        TO[64:, :, 8:16] = ev[:, 1:PAIRS + 1]
        W = (kp[l0:l0 + P_CORE]
             .reshape(PAIRS, 2, K, C, F)
             .transpose(1, 3, 0, 2, 4)
             .reshape(128, PAIRS, K, F))  # [pc, pair, k, f]
        blk = np.empty((128, TOT_COLS), np.float32)
        s = 0
        for h, n in enumerate(BLOCKS):
            o = BLK_OFF[h]
            w_cols = n * K * F
            blk[:, o:o + w_cols] = W[:, s:s + n].reshape(128, w_cols)
            blk[:, o + w_cols:o + w_cols + (n + 1) * 16] = (
                TE[:, s:s + n + 1].reshape(128, (n + 1) * 16))
            blk[:, o + w_cols + (n + 1) * 16:o + _blk_cols(n)] = (
                TO[:, s:s + n].reshape(128, n * 16))
            s += n
        in_maps.append({"blk": blk.astype(NPDT)})
    return in_maps


def _unpack_out(res):
    """(16, 32*512) per core -> (B, P_CORE, F).  l_local = 16g + 2j + phase."""
    return (res.reshape(2, 8, 32, 8, 64)          # [phase, b, g, j, f]
            .transpose(1, 2, 3, 0, 4)              # [b, g, j, phase, f]
            .reshape(B, P_CORE, F))


def kernel(x, kernel, bias):
    x = np.asarray(x, dtype=np.float32)
    kern = np.asarray(kernel, dtype=np.float32)
    bias = np.asarray(bias, dtype=np.float32)

    if "nc" not in _CACHE:
        _CACHE["nc"] = _build_nc()
    nc = _CACHE["nc"]

    in_maps = _prep_inputs(x, kern)
    results = run_bass_kernel_spmd(nc, in_maps, list(range(N_CORES))).results

    parts = [_unpack_out(results[m]["out"]) for m in range(N_CORES)]
    out = np.concatenate(parts, axis=1)[:, :L_OUT]
    return (out + bias[None]).astype(np.float32)
